# revision 17
# baseline (speedup 1.0000x reference)
"""Inverse STFT (nn_InverseSTFT) as a Bass/Tile kernel on 8 TRN2 NeuronCores.

Math
----
Reference computes, per batch b:
  full spectrum from one-sided stft via conjugate symmetry (F = 1024),
  ytmp[w, t] = sum_{f,c} full[f, t, c] * basis[f, w, c]          (IDFT)
  y = overlap_add(ytmp, hop=256), window-sum normalize, trim n_fft//2.

Folding the conjugate symmetry into the basis gives an exact K=1024 real
matmul (the imaginary basis rows for f=0 and f=512 are identically zero):
  rows 0..512   : A[f, w]  = cos-basis[f, w] + cos-basis[1024-f, w]   (f=1..511)
  rows 513..1023: Bm[f, w] = im-basis[f, w] - im-basis[1024-f, w]     (f=1..511)
computed with the reference's exact float32 angle arithmetic.

Since hop = 1024/4, write w = 256*j + r. Output sample n = 256*s + r:
  y[256 s + r] = sum_{j=0..3} sum_k basis[k, 256 j + r] * x[k, s - j]
The overlap-add is just PSUM accumulation over 4 frame-shifted matmuls.
Window-sum normalization = multiply by 1/(# valid j), which is 0.25 for
all output segments except s=2 (1/3), s=2000 (1/3), s=2001 (1/2), s=2002 (1).
Output keeps segments s = 2..2002 (trim = first 2 segments).

Sharding: pure data parallel, 2 batches per core.
"""

import numpy as np

import concourse.bass as bass
import concourse.mybir as mybir
from concourse.tile import TileContext
from concourse import bacc, bass_utils

N_FFT = 1024
HOP = 256
B = 16
NFREQ = 513
T = 2000
NCORES = 8
NB = B // NCORES          # batches per core
KC = 8                    # K chunks of 128 (K = 1024)
PAD_L = 3                 # left zero pad (j shifts up to 3)
TPAD = 2056               # 3 + 2000 + 53 (right pad covers last tile reads)
SEG = 2003                # total segments in un-trimmed output
OUT_SEGS = 2001           # segments s = 2..2002
NT = 16                   # s-tiles of 128 per batch (last has 81 valid rows)
OUT_LEN = OUT_SEGS * HOP  # 512256

F32 = mybir.dt.float32

# Matmul input dtype: bfloat16 halves stft/basis DMA traffic and enables
# fast weight load (FWL) on the PE; accumulation stays fp32 in PSUM.
# Validated rel-err vs reference: f32 1.6e-6, bf16 2.1e-3.
import os as _os

USE_BF16 = _os.environ.get("ISTFT_BF16", "1") == "1"
DT_IN = mybir.dt.bfloat16 if USE_BF16 else F32

import ml_dtypes

NP_IN = ml_dtypes.bfloat16 if USE_BF16 else np.float32


def _make_basis() -> np.ndarray:
    """(1024, 1024) folded basis, matching reference's float32 angle math."""
    f = np.arange(N_FFT, dtype=np.float32)
    w = np.arange(N_FFT, dtype=np.float32)
    a32 = np.float32(2.0 * np.pi / N_FFT)
    t1 = (a32 * f).astype(np.float32)
    ang = (t1[:, None] * w[None, :]).astype(np.float32)
    reb = (np.cos(ang).astype(np.float32) / np.float32(N_FFT)).astype(np.float32)
    imb = (-np.sin(ang).astype(np.float32) / np.float32(N_FFT)).astype(np.float32)
    A = np.empty((NFREQ, N_FFT), np.float32)
    A[0] = reb[0]
    A[512] = reb[512]
    A[1:512] = reb[1:512] + reb[1023:512:-1]
    Bm = (imb[1:512] - imb[1023:512:-1]).astype(np.float32)
    return np.concatenate([A, Bm], axis=0)


def _make_scales() -> np.ndarray:
    """(128, 2) per-partition wss fixup (on top of the 0.25 folded into basis).

    col 0 -> first s-tile (s = 2..129): s=2 has 3 frames -> 4/3.
    col 1 -> last s-tile (s = 1922..2002): s=2000 -> 4/3, 2001 -> 2, 2002 -> 4.
    """
    sc = np.ones((128, 2), np.float32)
    sc[0, 0] = np.float32(4.0) / np.float32(3.0)
    sc[78, 1] = np.float32(4.0) / np.float32(3.0)
    sc[79, 1] = 2.0
    sc[80, 1] = 4.0
    return sc


def _prep_x(stft: np.ndarray) -> np.ndarray:
    """(16,513,2000,2) f32 -> (16, KC, 128, TPAD) K-major, t zero-padded."""
    re = stft[:, :, :, 0]                  # (B, 513, T)
    im = stft[:, 1:512, :, 1]              # (B, 511, T)
    xk = np.concatenate([re, im], axis=1)  # (B, 1024, T)
    X = np.zeros((B, N_FFT, TPAD), np.float32)
    X[:, :, PAD_L : PAD_L + T] = xk
    return np.ascontiguousarray(X.reshape(B, KC, 128, TPAD))


def _build_nc() -> bass.Bass:
    nc = bacc.Bacc()
    x_in = nc.dram_tensor("x_in", [NB, KC, 128, TPAD], DT_IN, kind="ExternalInput")
    basis_in = nc.dram_tensor("basis_in", [KC, 128, N_FFT], DT_IN, kind="ExternalInput")
    scale_in = nc.dram_tensor("scale_in", [128, 2], F32, kind="ExternalInput")
    out = nc.dram_tensor("out", [NB, OUT_SEGS, HOP], F32, kind="ExternalOutput")

    with TileContext(nc) as tc:
        with (
            tc.tile_pool(name="xp", bufs=1) as x_pool,
            tc.tile_pool(name="bp", bufs=1) as b_pool,
            tc.tile_pool(name="sp", bufs=1) as s_pool,
            tc.tile_pool(name="ev", bufs=4) as ev_pool,
            tc.tile_pool(name="ps", bufs=4, space="PSUM") as psum_pool,
        ):
            # x chunks issue first on the Sync HWDGE queues (the first
            # matmul's critical path); basis + scale go via GpSimd so the
            # two DMA instruction streams issue in parallel.
            x_sb = [[None] * KC for _ in range(NB)]
            for b in range(NB):
                for kc in range(KC):
                    xt = x_pool.tile(
                        [128, TPAD], DT_IN, name=f"x{b}_{kc}", tag=f"x{b}_{kc}"
                    )
                    # split columns across two queue slots so the leading
                    # columns (first s-tiles' weights) land sooner
                    h = TPAD // 2
                    nc.sync.dma_start(xt[:, :h], x_in[b, kc, :, :h])
                    nc.sync.dma_start(xt[:, h:], x_in[b, kc, :, h:])
                    x_sb[b][kc] = xt

            basis_sb = []
            for kc in range(KC):
                bt = b_pool.tile([128, N_FFT], DT_IN, name=f"bas{kc}", tag=f"bas{kc}")
                nc.gpsimd.dma_start(bt[:, :], basis_in[kc])
                basis_sb.append(bt)

            scale_sb = s_pool.tile([128, 2], F32, name="scale_sb", tag="scale_sb")
            scale_wu = s_pool.tile([128, 2], F32, name="scale_wu", tag="scale_wu")
            nc.gpsimd.dma_start(scale_sb[:, :], scale_in[:, :])
            # ACT warm-up read of the scale table so later edge-tile
            # activations on ScalarE don't each need the DMA-sem wait.
            nc.scalar.copy(scale_wu[:, :], scale_sb[:, :])

            for b in range(NB):
                for st in range(NT):
                    s0 = 2 + 128 * st
                    psum = psum_pool.tile([128, HOP], F32, name="psum", tag="psum")
                    first = True
                    for kc in range(KC):
                        for j in range(4):
                            c0 = s0 - j + PAD_L
                            nc.tensor.matmul(
                                psum[:, :],
                                x_sb[b][kc][:, c0 : c0 + 128],
                                basis_sb[kc][:, HOP * j : HOP * (j + 1)],
                                start=first,
                                stop=(kc == KC - 1 and j == 3),
                            )
                            first = False
                    # basis is pre-scaled by 0.25 (the steady-state 1/wss);
                    # the two edge tiles apply a per-partition fixup scale
                    # via ScalarE's activation scale vector.
                    ev = ev_pool.tile([128, HOP], F32, name="ev", tag="ev")
                    if st == 0:
                        nc.scalar.mul(ev[:, :], psum[:, :], scale_sb[:, 0:1])
                    elif st == NT - 1:
                        nc.scalar.mul(ev[:, :], psum[:, :], scale_sb[:, 1:2])
                    else:
                        nc.vector.tensor_copy(ev[:, :], psum[:, :])
                    rows = min(128, SEG - s0)
                    nc.sync.dma_start(
                        out[b, 128 * st : 128 * st + rows, :], ev[:rows, :]
                    )
    nc.finalize()
    return nc


def _run(inputs: dict, trace: bool = False):
    stft = np.asarray(inputs["stft_matrix"], dtype=np.float32)
    X = np.ascontiguousarray(_prep_x(stft).astype(NP_IN))
    basis = np.ascontiguousarray(
        (_make_basis() * np.float32(0.25)).reshape(KC, 128, N_FFT).astype(NP_IN)
    )

    scales = _make_scales()
    in_maps = [
        {"x_in": X[NB * c : NB * (c + 1)], "basis_in": basis, "scale_in": scales}
        for c in range(NCORES)
    ]
    nc = _build_nc()
    res = bass_utils.run_bass_kernel_spmd(
        nc, in_maps, core_ids=list(range(NCORES)), trace=trace
    )
    out = np.concatenate(
        [res.results[c]["out"].reshape(NB, OUT_LEN) for c in range(NCORES)], axis=0
    )
    return out, res


def kernel(**inputs) -> np.ndarray:
    out, _ = _run(inputs, trace=False)
    return out
